# revision 36
# baseline (speedup 1.0000x reference)
"""BitNetV3Attention (B=2, S=2048, H=16, DH=128, D=2048) on 8 TRN2 NeuronCores.

Strategy (tensor-parallel over heads + row-parallel o_proj):
  - Each core owns 2 of 16 heads. It computes Q^T/K^T (head-transposed,
    [DH, B*S]) and V ([B*S, DH]) for its heads from the full hidden states
    (replicated read), runs causal flash-style attention per (head, batch),
    producing normalized attn_out^T slices [256, B*S].
  - Two AllToAll collectives (one per local head slot) redistribute attn_out
    from head-sharded to sequence-sharded: core j ends with
    attn_out^T[:, rows_j] for ALL 2048 model dims, where rows_j are 512 rows
    of the [4096, 2048] token matrix. The first A2A fires after local head 0
    finishes and overlaps head 1's attention.
  - Each core computes its 512 output rows against the full Wo (no
    all-reduce needed; outputs concatenate on host). o_proj accumulates
    even d-tiles (from A2A#0) before odd ones (A2A#1) so it can start
    before the second collective lands. Wo slabs prefetch during attention.

All matmuls run in bf16 (same 1 col/cycle PE rate as float32r but no
small-tile fp32r penalty, half the HBM/SBUF traffic; ~4e-3 relative error).
Softmax skips max-subtraction (scores are O(5), exp is safe); the padding
mask rides the ScalarE activation's per-partition bias; the causal mask is
a single [128, 1024] additive template sliced per diagonal tile.

o_proj runs in two passes: pass A accumulates the even d-tiles (available
after A2A#0) into SBUF partials — PE work that hides A2A#1's latency —
and pass B adds the odd d-tiles once A2A#1 lands.
"""
import sys
for _p in ('/opt/trn_rl_repo', '/root/.axon_site/_ro/trn_rl_repo'):
    if _p not in sys.path:
        sys.path.append(_p)

import numpy as np

import concourse.mybir as mybir
import concourse.tile as tile
from concourse import bacc, bass_utils

B, S, H, DH = 2, 2048, 16, 128
D = H * DH                  # 2048
NS = B * S                  # 4096
NC = 8                      # cores
HL = H // NC                # 2 local heads
DSL = HL * DH               # 256 (d-slice per core)
ROWS = NS // NC             # 512 output rows per core
SCALE = 1.0 / float(np.sqrt(DH))
F32 = mybir.dt.float32
F32R = mybir.dt.float32r
BF16 = mybir.dt.bfloat16
# dtype used for QKV/attention matmul operands (F32R or BF16) - A/B testable
MM_DT = BF16
EXP = mybir.ActivationFunctionType.Exp
NEG = -1.0e30

N_K = D // 128              # 16 contraction tiles
N_SC = NS // 512            # 8 s-chunks for QKV
N_QC = S // 512             # 4 q-chunks per batch

# attention pipeline depths (PSUM: PS+PO+PD <= 8 banks)
PS_BUFS, PO_BUFS, PD_BUFS = 4, 2, 2
EX_BUFS, AO_BUFS, REC_BUFS = 10, 4, 4
# split big steady-state loads across both HWDGE queues (SP + ACT) so two
# SDMA engines run concurrently on real hardware
SPLIT_QUEUES = True


def build_bass(repeat=1, do_attn=True, do_a2a=True, do_oproj=True):
    nc = bacc.Bacc("TRN2", target_bir_lowering=False, debug=False, num_devices=NC)

    ht = nc.dram_tensor("ht", [D, NS], MM_DT, kind="ExternalInput").ap()
    wqt = nc.dram_tensor("wqt", [D, DSL], MM_DT, kind="ExternalInput").ap()
    wkt = nc.dram_tensor("wkt", [D, DSL], MM_DT, kind="ExternalInput").ap()
    wvt = nc.dram_tensor("wvt", [D, DSL], MM_DT, kind="ExternalInput").ap()
    wot = nc.dram_tensor("wot", [D, D], BF16, kind="ExternalInput").ap()
    pad = nc.dram_tensor("pad", [B, S], F32, kind="ExternalInput").ap()
    tri = nc.dram_tensor("tri", [128, 1024], F32, kind="ExternalInput").ap()
    onesd = nc.dram_tensor("ones", [128, 128], MM_DT, kind="ExternalInput").ap()
    idend = nc.dram_tensor("iden", [128, 128], MM_DT, kind="ExternalInput").ap()
    out = nc.dram_tensor("out", [ROWS, D], F32, kind="ExternalOutput").ap()

    with tile.TileContext(nc) as tc:
        with tc.tile_pool(name="dram", bufs=1, space="DRAM") as dram, \
             tc.tile_pool(name="const", bufs=1) as cpool:
            a2a_in = [dram.tile([NC, DH, 512], BF16, name=f"a2a_in{h}") for h in range(HL)]
            a2a_out = [dram.tile([NC, DH, 512], BF16, name=f"a2a_out{h}") for h in range(HL)]

            tri_sb = cpool.tile([128, 1024], F32)
            pad_sb = cpool.tile([128, B * 16], F32)
            ones_sb = cpool.tile([128, 128], MM_DT)
            iden_sb = cpool.tile([128, 128], MM_DT)

            def const_dmas():
                # emitted mid-startup (after the QKV weight loads) so they
                # stay off the critical path: iden is first needed at the
                # sc=0 V-transposes, tri/pad only at attention time.
                nc.sync.dma_start(iden_sb[:], idend)
                nc.sync.dma_start(tri_sb[:], tri)
                nc.sync.dma_start(
                    pad_sb[:].rearrange("p (b t) -> p b t", b=B),
                    pad.rearrange("b (t p) -> p b t", p=128),
                )
                nc.sync.dma_start(ones_sb[:], onesd)

            for _rep in range(repeat):
                _emit_body(nc, tc, a2a_in, a2a_out, tri_sb, pad_sb, ones_sb,
                           iden_sb, ht, wqt, wkt, wvt, wot, out,
                           do_attn=do_attn, do_a2a=do_a2a, do_oproj=do_oproj,
                           const_dmas=const_dmas if _rep == 0 else None)
    nc.compile()
    return nc


def _emit_qkv(nc, tc, qt_sb, kt_sb, v_sb, iden_sb, ht, wqt, wkt, wvt,
              const_dmas=None):
    with tc.tile_pool(name="wts", bufs=1) as wpool, \
         tc.tile_pool(name="hts", bufs=3) as hpool, \
         tc.tile_pool(name="vtt", bufs=2) as vpool, \
         tc.tile_pool(name="ps1", bufs=1, space="PSUM") as pp1:
        w_srcs = (("q", wqt), ("k", wkt), ("v", wvt))
        w_sb = {nm: wpool.tile([128, N_K * DSL], MM_DT, name=f"w{nm}")
                for nm, _ in w_srcs}

        def issue_w(klo, khi, eng=nc.sync):
            for nm, src in w_srcs:
                dst = w_sb[nm][:, DSL*klo:DSL*khi]
                if khi - klo > 1:
                    dst = dst.rearrange("p (t m) -> p t m", t=khi-klo)
                    eng.dma_start(
                        dst,
                        src[128*klo:128*khi, :].rearrange("(t p) m -> p t m", p=128))
                else:
                    eng.dma_start(dst, src[128*klo:128*khi, :])

        ht_r = ht.rearrange("(k p) s -> p k s", p=128)

        def issue_slab(sc, half, split=False):
            slab = hpool.tile([128, 8 * 512], MM_DT, tag="ht", name="htslab")
            pieces = ((0, 2), (2, 8)) if split else ((0, 8),)
            for klo, khi in pieces:
                nc.sync.dma_start(
                    slab[:, 512*klo:512*khi].rearrange(
                        "p (k s) -> p k s", k=khi-klo),
                    ht_r[:, 8*half+klo:8*half+khi, 512*sc:512*sc+512])
            return slab

        # critical-path-ordered startup: k=0 weights + first quarter-slab
        # first so the first matmuls start a few us in; remaining weights
        # arrive in k-chunks just ahead of the PE's contraction sweep,
        # alternating between the two HWDGE queues so copies overlap.
        issue_w(0, 1)
        s00 = issue_slab(0, 0, split=True)
        issue_w(1, 5)
        issue_w(5, 9)
        s01 = issue_slab(0, 1)
        issue_w(9, 13)
        issue_w(13, 16)
        if const_dmas is not None:
            const_dmas()
        slabs0 = [s00, s01]

        # PE-transpose V chunk to natural [s, dh] layout; deferred one
        # (sc, h) step so the transposes never stall PE on the psvt drain.
        def emit_transposes(sc, h, vt):
            for m in range(4):
                ptp = pp1.tile([128, 128], MM_DT, tag="ptp", name="ptp",
                               bufs=2)
                nc.tensor.transpose(
                    ptp[:], vt[:, 128*m:128*m+128], iden_sb[:])
                st = 4 * sc + m
                if (h + m) % 2 == 0:
                    nc.vector.tensor_copy(
                        v_sb[h][:, 128*st:128*st+128], ptp[:])
                else:
                    nc.scalar.copy(
                        v_sb[h][:, 128*st:128*st+128], ptp[:])

        def issue_slab_split(sc, half):
            # one half-slab per HWDGE queue → both halves copy concurrently
            slab = hpool.tile([128, 8 * 512], MM_DT, tag="ht", name="htslab")
            for eng, klo, khi in ((nc.sync, 0, 4), (nc.scalar, 4, 8)):
                eng.dma_start(
                    slab[:, 512*klo:512*khi].rearrange(
                        "p (k s) -> p k s", k=khi-klo),
                    ht_r[:, 8*half+klo:8*half+khi, 512*sc:512*sc+512])
            return slab

        mk_slab = issue_slab_split if SPLIT_QUEUES else issue_slab
        pending = None
        for sc in range(N_SC):
            slabs = slabs0 if sc == 0 else [mk_slab(sc, 0), mk_slab(sc, 1)]
            # heads sequentially: halves live PSUM so q/k/v accumulators can
            # double-buffer — drains overlap the next head's matmuls.
            for h in range(HL):
                psq = pp1.tile([128, 512], F32, tag="pq", name="pq", bufs=2)
                psk = pp1.tile([128, 512], F32, tag="pk", name="pk", bufs=2)
                psvt = pp1.tile([128, 512], F32, tag="pvt", name="pvt", bufs=2)
                for k in range(N_K):
                    htt = slabs[k // 8][:, 512*(k % 8):512*(k % 8)+512]
                    fl = dict(start=(k == 0), stop=(k == N_K - 1))
                    nc.tensor.matmul(
                        psq[:], w_sb["q"][:, DSL*k+128*h:DSL*k+128*h+128],
                        htt, **fl)
                    nc.tensor.matmul(
                        psk[:], w_sb["k"][:, DSL*k+128*h:DSL*k+128*h+128],
                        htt, **fl)
                    nc.tensor.matmul(
                        psvt[:], w_sb["v"][:, DSL*k+128*h:DSL*k+128*h+128],
                        htt, **fl)
                # drain PSUM -> SBUF, split across DVE and ACT
                nc.vector.tensor_copy(qt_sb[h][:, 512*sc:512*sc+512], psq[:])
                nc.scalar.copy(kt_sb[h][:, 512*sc:512*sc+512], psk[:])
                vt = vpool.tile([128, 512], MM_DT, tag="vtt", name="vtt",
                                bufs=2)
                if h == 0:
                    nc.vector.tensor_copy(vt[:], psvt[:])
                else:
                    nc.scalar.copy(vt[:], psvt[:])
                if pending is not None:
                    emit_transposes(*pending)
                pending = (sc, h, vt)
        emit_transposes(*pending)


def _emit_attention(nc, tc, qt_sb, kt_sb, v_sb, tri_sb, pad_sb, ones_sb,
                    a2a_in, a2a_out, do_a2a, post_collective=None):
    with tc.tile_pool(name="att", bufs=1) as apool, \
         tc.tile_pool(name="ps2", bufs=1, space="PSUM") as pp2:
        for h in range(HL):
            for b in range(B):
                for qc in range(N_QC):
                    q0 = 512 * qc
                    n_sk = 4 * qc + 4
                    po = pp2.tile([128, 512], F32, tag="po", bufs=PO_BUFS, name="po")
                    pd = pp2.tile([128, 512], F32, tag="pd", bufs=PD_BUFS, name="pd")
                    for t in range(n_sk):
                        # columns sq < o are fully causal-masked; skip them
                        o = max(0, 128 * t - q0)
                        ps = pp2.tile([128, 512], F32, tag="ps", bufs=PS_BUFS, name="ps")
                        nc.tensor.matmul(
                            ps[:, o:512],
                            kt_sb[h][:, S*b+128*t:S*b+128*t+128],
                            qt_sb[h][:, S*b+q0+o:S*b+q0+512],
                            start=True, stop=True)
                        if t >= 4 * qc:  # diagonal block
                            nc.vector.tensor_add(
                                ps[:, o:512], ps[:, o:512], tri_sb[:, 512:1024-o])
                        ex = apool.tile([128, 512], MM_DT, tag="ex", bufs=EX_BUFS, name="ex")
                        nc.scalar.activation(
                            ex[:, o:512], ps[:, o:512], EXP,
                            bias=pad_sb[:, 16*b+t:16*b+t+1], scale=SCALE)
                        fl = dict(start=(t == 0), stop=(t == n_sk - 1))
                        st = 16 * b + t
                        nc.tensor.matmul(
                            po[:, o:512], v_sb[h][:, 128*st:128*st+128],
                            ex[:, o:512], **fl)
                        nc.tensor.matmul(
                            pd[:, o:512], ones_sb[:], ex[:, o:512], **fl)
                    rec = apool.tile([128, 512], F32, tag="rec", bufs=REC_BUFS, name="rec")
                    nc.vector.reciprocal(rec[:], pd[:])
                    ao = apool.tile([128, 512], BF16, tag="ao", bufs=AO_BUFS, name="ao")
                    nc.vector.tensor_mul(ao[:], po[:], rec[:])
                    nc.sync.dma_start(a2a_in[h][4*b+qc, :, :], ao[:])
            # ---- AllToAll for this head-slot (overlaps next head's attn) ----
            if do_a2a:
                nc.gpsimd.collective_compute(
                    "AllToAll", mybir.AluOpType.bypass,
                    replica_groups=[list(range(NC))],
                    ins=[a2a_in[h].opt()], outs=[a2a_out[h].opt()])
            if post_collective is not None:
                post_collective[h]()


def _emit_oproj(nc, tc, at_sb, wopool, obpool, papool, load_wo, wo_slabs,
                out):
    # Two passes over the 16 output tiles: pass A accumulates the even
    # d-tiles (from A2A#0) into SBUF partials — 27us of PE work that runs
    # entirely under A2A#1's latency; pass B accumulates the odd d-tiles and
    # merges. Global d-tile g lives at a2a_out[g % 2][g // 2].
    with tc.tile_pool(name="ps4", bufs=4, space="PSUM") as pp4:
        part = papool.tile([128, 16 * 512], F32, name="part")
        # prefetch the odd-half Wo slabs during pass A
        for ne in range(4):
            if (ne, 1) not in wo_slabs:
                load_wo(ne, 1)
        for ne in range(4):
            if (ne, 0) not in wo_slabs:
                load_wo(ne, 0)
            slab = wo_slabs[(ne, 0)]
            for m in range(4):
                pout = pp4.tile([128, 512], F32, tag="pout", name="pout", bufs=4)
                for i in range(8):
                    nc.tensor.matmul(
                        pout[:],
                        at_sb[0][:, 512*i+128*m:512*i+128*m+128],
                        slab[:, 512*i:512*i+512],
                        start=(i == 0), stop=(i == 7))
                sl = 512 * (4 * ne + m)
                if (ne + m) % 2 == 0:
                    nc.vector.tensor_copy(part[:, sl:sl+512], pout[:])
                else:
                    nc.scalar.copy(part[:, sl:sl+512], pout[:])
        for ne in range(4):
            slab = wo_slabs[(ne, 1)]
            for m in range(4):
                pout = pp4.tile([128, 512], F32, tag="pout", name="pout", bufs=4)
                for i in range(8):
                    nc.tensor.matmul(
                        pout[:],
                        at_sb[1][:, 512*i+128*m:512*i+128*m+128],
                        slab[:, 512*i:512*i+512],
                        start=(i == 0), stop=(i == 7))
                sl = 512 * (4 * ne + m)
                ob = obpool.tile([128, 512], F32, tag="ob", name="ob",
                                 bufs=4)
                nc.vector.tensor_add(ob[:], pout[:], part[:, sl:sl+512])
                nc.sync.dma_start(
                    out[128*m:128*m+128, 512*ne:512*ne+512], ob[:])


def _emit_body(nc, tc, a2a_in, a2a_out, tri_sb, pad_sb, ones_sb,
               iden_sb, ht, wqt, wkt, wvt, wot, out,
               do_attn=True, do_a2a=True, do_oproj=True, const_dmas=None):
    with tc.tile_pool(name="store", bufs=1) as spool:
        qt_sb = [spool.tile([128, NS], MM_DT, name=f"qt{h}") for h in range(HL)]
        kt_sb = [spool.tile([128, NS], MM_DT, name=f"kt{h}") for h in range(HL)]
        v_sb = [spool.tile([128, NS], MM_DT, name=f"v{h}") for h in range(HL)]

        _emit_qkv(nc, tc, qt_sb, kt_sb, v_sb, iden_sb, ht, wqt, wkt, wvt,
                  const_dmas=const_dmas)

        # o_proj pools open before attention so Wo/at_sb DMAs can prefetch
        # into the space vacated by the QKV weight/ht pools during attention.
        # Loads go on the ACT hwdge queue, decoupled from the SP queue that
        # carries the attention a2a stores.
        with tc.tile_pool(name="oproj", bufs=1) as opool, \
             tc.tile_pool(name="wo", bufs=2) as wopool, \
             tc.tile_pool(name="ob", bufs=3) as obpool, \
             tc.tile_pool(name="part", bufs=1) as papool:
            at_sb = [opool.tile([128, 8 * 512], BF16, name=f"at{half}")
                     for half in range(2)]

            def load_at(half):
                if SPLIT_QUEUES:
                    # halve the critical post-collective load across queues
                    for eng, jlo, jhi in ((nc.scalar, 0, 4), (nc.sync, 4, 8)):
                        eng.dma_start(
                            at_sb[half][:, 512*jlo:512*jhi].rearrange(
                                "p (j s) -> p j s", j=jhi-jlo),
                            a2a_out[half][jlo:jhi].rearrange("j p s -> p j s"))
                else:
                    nc.scalar.dma_start(
                        at_sb[half][:].rearrange("p (j s) -> p j s", j=8),
                        a2a_out[half].rearrange("j p s -> p j s"))

            # wot rows (t p) with t = global d-tile; split parity for slabs
            wot_r2 = wot.rearrange("(t2 two p) e -> p two t2 e", p=128, two=2)
            wo_slabs = {}

            def load_wo(ne, half):
                sl = wopool.tile([128, 8 * 512], BF16, tag=f"wo{half}",
                                 name=f"wo{half}", bufs=4)
                nc.scalar.dma_start(
                    sl[:].rearrange("p (t e) -> p t e", t=8),
                    wot_r2[:, half, :, 512*ne:512*ne+512])
                wo_slabs[(ne, half)] = sl

            def post_c0():
                # fires once A2A#0 lands, overlapping head 1's attention
                load_at(0)
                load_wo(0, 0)
                load_wo(0, 1)

            post_collective = [post_c0, lambda: load_at(1)]

            if do_attn:
                _emit_attention(nc, tc, qt_sb, kt_sb, v_sb, tri_sb, pad_sb,
                                ones_sb, a2a_in, a2a_out, do_a2a,
                                post_collective=post_collective)
            if do_oproj:
                if not do_attn:
                    post_c0()
                    load_at(1)
                _emit_oproj(nc, tc, at_sb, wopool, obpool, papool, load_wo,
                            wo_slabs, out)


_NC_CACHE = None


def _get_nc():
    global _NC_CACHE
    if _NC_CACHE is None:
        _NC_CACHE = build_bass()
    return _NC_CACHE


def make_in_maps(hidden_states, attention_mask, Wq, Wk, Wv, Wo):
    import ml_dtypes
    mm_np = np.float32 if MM_DT == F32R else ml_dtypes.bfloat16
    x = np.ascontiguousarray(np.asarray(hidden_states, dtype=np.float32)).reshape(NS, D)
    ht = np.ascontiguousarray(x.T).astype(mm_np)                     # [D, NS]
    wqt = np.ascontiguousarray(np.asarray(Wq, dtype=np.float32).T).astype(mm_np)
    wkt = np.ascontiguousarray(np.asarray(Wk, dtype=np.float32).T).astype(mm_np)
    wvt = np.ascontiguousarray(np.asarray(Wv, dtype=np.float32).T).astype(mm_np)
    import ml_dtypes
    wot = np.ascontiguousarray(
        np.asarray(Wo, dtype=np.float32).T).astype(ml_dtypes.bfloat16)
    mask = np.asarray(attention_mask)
    pad = np.where(mask == 0, np.float32(NEG), np.float32(0.0)).astype(np.float32)
    tri = np.where(
        np.arange(1024, dtype=np.int64)[None, :] >= np.arange(128, dtype=np.int64)[:, None] + 512,
        np.float32(0.0), np.float32(NEG)).astype(np.float32)
    ones = np.ones((128, 128), dtype=np.float32)
    iden = np.eye(128, dtype=np.float32)

    in_maps = []
    for c in range(NC):
        sl = slice(DSL * c, DSL * c + DSL)
        in_maps.append({
            "ht": ht,
            "wqt": np.ascontiguousarray(wqt[:, sl]),
            "wkt": np.ascontiguousarray(wkt[:, sl]),
            "wvt": np.ascontiguousarray(wvt[:, sl]),
            "wot": wot,
            "pad": pad,
            "tri": tri,
            "ones": ones.astype(mm_np),
            "iden": iden.astype(mm_np),
        })
    return in_maps


def assemble_output(results):
    rows = np.concatenate([results[c]["out"] for c in range(NC)], axis=0)
    return rows.reshape(B, S, D).astype(np.float32)


def kernel(hidden_states, attention_mask, Wq, Wk, Wv, Wo):
    nc = _get_nc()
    in_maps = make_in_maps(hidden_states, attention_mask, Wq, Wk, Wv, Wo)
    res = bass_utils.run_bass_kernel_spmd(nc, in_maps, core_ids=list(range(NC)))
    return assemble_output(res.results)

